# revision 13
# baseline (speedup 1.0000x reference)
"""DockingScorePredictor Trainium2 kernel.

Data-parallel over complexes: 8 cores, one complex (512 protein x 64 ligand
atoms) per core.  Only pairs inside the 8A cutoff matter (~43%), so the host
compacts valid pairs into a dense stream of NT tiles x 512 pairs.

Per pair the layer-1 preactivation is
  z1 = W1a.T hp + W1b.T hl + b1 + W1c.T rb(d)
and hp/hl depend only on the atom *type* (20/16 types), so z1 collapses to a
single K=68 matmul against [rb(32); onehot_ptype(20); onehot_ltype(16)] with
a fused weight [W1c; prot_emb@W1a; lig_emb@W1b + b1].  Device work per tile:
3 matmuls (K=68 z1, W2, W3; float32r, N=512, 1 col/cycle) + 3 relu
evacuations split DVE/ACT (~55%/45% of relu3 to balance engines), with the
pair-sum fused into the accum_out port.  A tiny N=256 filler matmul per
step keeps the PE's HAM clock-gate at 2.4 GHz (micro-idle gaps would
throttle it to 1.2 GHz and make the PE the bottleneck).

The device emits per-tile channel sums (acc) plus one guaranteed-pad
column's activation (cvec); the host removes the pad contribution
(tot - npad*cvec, exact in fp32), divides by the pair count, and runs the
5-flop-per-complex scoring head in fp32 during unsharding.
"""
import numpy as np
from contextlib import ExitStack

import concourse.bass as bass
import concourse.bacc as bacc
import concourse.tile as tile
from concourse import mybir
from concourse import bass_utils

F32 = mybir.dt.float32
F32R = mybir.dt.float32r
AF = mybir.ActivationFunctionType
ALU = mybir.AluOpType

B, P, L = 8, 512, 64
H, RB = 128, 32
NPT, NLT = 20, 16
CUTOFF = 8.0
N_CORES = 8
K1 = RB + NPT + NLT          # 68: contraction dim of the fused z1 matmul
NT_DEFAULT = 31              # tiles of 512 pairs
CHUNK = 4                    # rhs tiles per dma_start
WIDTH = 0.5 * CUTOFF / RB + 1e-8

_CACHE = {}


def _build_nc(nt):
    nc = bacc.Bacc("TRN2", target_bir_lowering=False, debug=False,
                   num_devices=N_CORES)
    d = {}

    def inp(name, shape, dt):
        d[name] = nc.dram_tensor(name, shape, dt, kind="ExternalInput").ap()

    inp("rhsG", [K1, nt * 512], F32R)
    inp("wa", [K1, H], F32R)      # W1z
    inp("wb", [H, 2 * H], F32R)   # W2 | W3
    inp("fb", [H, 2], F32)        # b2 | b3

    acc_ap = nc.dram_tensor("acc", [H, nt], F32, kind="ExternalOutput").ap()
    cvec_ap = nc.dram_tensor("cvec", [H, 1], F32, kind="ExternalOutput").ap()

    with tile.TileContext(nc) as tc:
        with ExitStack() as ctx:
            const = ctx.enter_context(tc.tile_pool(name="const", bufs=1))
            rbuf = ctx.enter_context(tc.tile_pool(name="rbuf", bufs=3))
            abuf = ctx.enter_context(tc.tile_pool(name="abuf", bufs=3))
            psZ1 = ctx.enter_context(tc.tile_pool(name="psZ1", bufs=3, space="PSUM"))
            psZ2 = ctx.enter_context(tc.tile_pool(name="psZ2", bufs=2, space="PSUM"))
            psZ3 = ctx.enter_context(tc.tile_pool(name="psZ3", bufs=2, space="PSUM"))
            psF = ctx.enter_context(tc.tile_pool(name="psF", bufs=1, space="PSUM"))

            nchunks = (nt + CHUNK - 1) // CHUNK
            PREFC = 2
            chunks, z1s, a1s, z2s, a2s, z3s = {}, {}, {}, {}, {}, {}

            def dma_chunk(c):
                lo = c * CHUNK * 512
                w = min(CHUNK * 512, nt * 512 - lo)
                rt = rbuf.tile([K1, CHUNK * 512], F32R, tag="rhs",
                               name=f"rhs{c}")
                nc.sync.dma_start(out=rt[:, 0:w], in_=d["rhsG"][:, lo:lo + w])
                chunks[c] = rt

            # first-tile weights land first so MM0 starts ASAP
            wa = const.tile([K1, H], F32R, tag="wa", name="wa")
            nc.sync.dma_start(out=wa, in_=d["wa"])
            dma_chunk(0)
            wb = const.tile([H, 2 * H], F32R, tag="wb", name="wb")
            nc.sync.dma_start(out=wb, in_=d["wb"])
            fb = const.tile([H, 2], F32, tag="fb", name="fb")
            nc.sync.dma_start(out=fb, in_=d["fb"])
            dma_chunk(1)
            W1z = wa[:, :]
            W2 = wb[:, 0:H]
            W3 = wb[:, H:2 * H]

            # warm the ACT table set before the pipeline needs relu
            warm = const.tile([1, 64], F32, tag="warm", name="warm")
            nc.vector.memset(warm[:, :], 0.0)
            nc.scalar.activation(out=warm[:, :], in_=warm[:, :], func=AF.Relu,
                                 bias=0.0, scale=1.0)
            # biases as dense [H,1] tiles
            b2 = const.tile([H, 1], F32, tag="b2", name="b2")
            nc.vector.tensor_scalar(out=b2[:, :], in0=fb[:, 0:1],
                                    scalar1=0.0, scalar2=None, op0=ALU.add)
            b3 = const.tile([H, 1], F32, tag="b3", name="b3")
            nc.vector.tensor_scalar(out=b3[:, :], in0=fb[:, 1:2],
                                    scalar1=0.0, scalar2=None, op0=ALU.add)

            acc = const.tile([H, nt], F32, tag="acc", name="acc")
            a3_last = const.tile([H, 512], F32, tag="a3_last", name="a3_last")
            zeros = const.tile([H, 512], F32, tag="zeros", name="zeros")
            nc.vector.memset(zeros[:, :], 0.0)

            # scratch bank for the HAM-warming filler matmuls
            fill_ps = psF.tile([H, 256], F32, tag="fill", name="fill_ps")

            # relu3 engine split: DVE for the tail (pad tiles round
            # identically to the extracted cvec) + 3/8 of the rest
            def r3_on_dve(tt):
                return (tt >= nt - 8) or (tt % 8 < 3)

            for step in range(nt + 6):
                t0 = step
                if t0 < nt:
                    c0, s0 = divmod(t0, CHUNK)
                    if s0 == 0 and c0 + PREFC < nchunks:
                        dma_chunk(c0 + PREFC)
                    z1 = psZ1.tile([H, 512], F32, tag="z1", name=f"z1_{t0}")
                    z1s[t0] = z1
                    if step > 0:
                        nc.tensor.matmul(out=fill_ps[:, :],
                                         lhsT=W1z, rhs=chunks[c0][:, 0:256],
                                         start=True, stop=True)
                    nc.tensor.matmul(out=z1[:, :], lhsT=W1z,
                                     rhs=chunks[c0][:, 512 * s0:512 * (s0 + 1)],
                                     start=True, stop=True)
                t1 = step - 1
                if 0 <= t1 < nt:
                    a1 = abuf.tile([H, 512], F32R, tag="a1", name=f"a1_{t1}")
                    a1s[t1] = a1
                    nc.vector.tensor_scalar(out=a1[:, :],
                                            in0=z1s.pop(t1)[:, :],
                                            scalar1=0.0, scalar2=None,
                                            op0=ALU.max)
                t2 = step - 2
                if 0 <= t2 < nt:
                    z2 = psZ2.tile([H, 512], F32, tag="z2", name=f"z2_{t2}")
                    z2s[t2] = z2
                    nc.tensor.matmul(out=z2[:, :], lhsT=W2,
                                     rhs=a1s.pop(t2)[:, :],
                                     start=True, stop=True)
                t3 = step - 3
                if 0 <= t3 < nt:
                    a2 = abuf.tile([H, 512], F32R, tag="a2", name=f"a2_{t3}")
                    a2s[t3] = a2
                    nc.scalar.activation(out=a2[:, :], in_=z2s.pop(t3)[:, :],
                                         func=AF.Relu, bias=b2, scale=1.0)
                t4 = step - 4
                if 0 <= t4 < nt:
                    z3 = psZ3.tile([H, 512], F32, tag="z3", name=f"z3_{t4}")
                    z3s[t4] = z3
                    nc.tensor.matmul(out=z3[:, :], lhsT=W3,
                                     rhs=a2s.pop(t4)[:, :],
                                     start=True, stop=True)
                t5 = step - 5
                if 0 <= t5 < nt:
                    if t5 == nt - 1:
                        a3 = a3_last
                    else:
                        a3 = abuf.tile([H, 512], F32R, tag="a3",
                                       name=f"a3_{t5}")
                    z3ap = z3s.pop(t5)
                    if r3_on_dve(t5):
                        # out = max(z3 + b3, 0); accum = sum(out)
                        nc.vector.scalar_tensor_tensor(
                            out=a3[:, :], in0=z3ap[:, :],
                            scalar=b3, in1=zeros[:, :],
                            op0=ALU.add, op1=ALU.max,
                            accum_out=acc[:, t5:t5 + 1])
                    else:
                        nc.scalar.activation(out=a3[:, :], in_=z3ap[:, :],
                                             func=AF.Relu, bias=b3,
                                             scale=1.0,
                                             accum_out=acc[:, t5:t5 + 1])

            nc.sync.dma_start(out=acc_ap, in_=acc[:, :])
            nc.sync.dma_start(out=cvec_ap, in_=a3_last[:, 511:512])

    nc.compile()
    return nc


def _get_nc(nt=NT_DEFAULT):
    if nt not in _CACHE:
        _CACHE[nt] = _build_nc(nt)
    return _CACHE[nt]


def kernel(protein_pos, ligand_pos, prot_emb, lig_emb,
           W1, b1, W2, b2, W3, b3, Wr1, br1, Wr2, br2,
           protein_atom_type, ligand_atom_type, protein_batch, ligand_batch):
    protein_pos = np.asarray(protein_pos, dtype=np.float32).reshape(B, P, 3)
    ligand_pos = np.asarray(ligand_pos, dtype=np.float32).reshape(B, L, 3)
    prot_emb = np.asarray(prot_emb, dtype=np.float32)
    lig_emb = np.asarray(lig_emb, dtype=np.float32)
    W1 = np.asarray(W1, dtype=np.float32)
    b1 = np.asarray(b1, dtype=np.float32)
    W2 = np.asarray(W2, dtype=np.float32)
    W3 = np.asarray(W3, dtype=np.float32)
    Wr1 = np.asarray(Wr1, dtype=np.float32)
    br1 = np.asarray(br1, dtype=np.float32).reshape(H)
    Wr2 = np.asarray(Wr2, dtype=np.float32).reshape(H)
    br2 = float(np.asarray(br2, dtype=np.float32).reshape(()))
    ptype = np.asarray(protein_atom_type).reshape(B, P)
    ltype = np.asarray(ligand_atom_type).reshape(B, L)

    # fused stage-1 weight: z1 = W1z.T @ [rb; onehot_p; onehot_l]
    PA = prot_emb @ W1[0:H, :]                      # [20, 128]
    LA = lig_emb @ W1[H:2 * H, :] + b1[None, :]     # [16, 128]
    W1z = np.ascontiguousarray(
        np.concatenate([W1[2 * H:2 * H + RB, :], PA, LA], axis=0),
        dtype=np.float32)
    wbb = np.ascontiguousarray(np.concatenate([W2, W3], axis=1),
                               dtype=np.float32)
    fbb = np.stack([np.asarray(b2, np.float32).reshape(H),
                    np.asarray(b3, np.float32).reshape(H)], axis=1)
    fbb = np.ascontiguousarray(fbb)

    centers = np.linspace(0.0, CUTOFF, RB, dtype=np.float32)

    # per-complex valid-pair extraction
    pis, lis, dists, cnts = [], [], [], []
    kmax = 0
    for b in range(B):
        diff = protein_pos[b][:, None, :] - ligand_pos[b][None, :, :]
        dist = np.sqrt((diff * diff).sum(-1, dtype=np.float32))
        pi, li = np.nonzero(dist < np.float32(CUTOFF))
        pis.append(pi); lis.append(li); dists.append(dist[pi, li])
        cnts.append(len(pi)); kmax = max(kmax, len(pi))

    nt = max(NT_DEFAULT, (kmax + 1 + 511) // 512)
    npair = nt * 512

    in_maps = []
    for b in range(B):
        pi, li, dv, cnt = pis[b], lis[b], dists[b], cnts[b]
        rhs = np.zeros((K1, npair), dtype=np.float32)
        rhs[0:RB, :cnt] = np.exp(
            -0.5 * ((dv[:, None] - centers[None, :]) / WIDTH) ** 2).T
        rhs[RB + ptype[b][pi], np.arange(cnt)] = 1.0
        rhs[RB + NPT + ltype[b][li], np.arange(cnt)] = 1.0
        in_maps.append({"rhsG": rhs, "wa": W1z, "wb": wbb, "fb": fbb})

    nc = _get_nc(nt)
    res = bass_utils.run_bass_kernel_spmd(nc, in_maps,
                                          core_ids=list(range(N_CORES)))

    # unshard: pad removal + mean + scoring head (fp32, ~5 flop/complex)
    out = np.zeros(B, dtype=np.float32)
    for b in range(B):
        cnt = cnts[b]
        acc = res.results[b]["acc"]          # [H, nt]
        cvec = res.results[b]["cvec"][:, 0]  # [H]
        tot = acc.sum(axis=1) - (npair - cnt) * cvec
        if cnt > 0:
            repr_ = (tot / cnt).astype(np.float32)
            r1 = np.maximum(repr_ @ Wr1 + br1, 0.0)
            out[b] = r1 @ Wr2 + br2
    return out


# revision 16
# speedup vs baseline: 1.0960x; 1.0960x over previous
"""DockingScorePredictor Trainium2 kernel.

Only pairs inside the 8A cutoff matter (~43%), so the host compacts valid
pairs into dense 512-pair tiles.  The per-pair layer-1 preactivation
  z1 = W1a.T hp + W1b.T hl + b1 + W1c.T rb(d)
depends on atom *types* (20/16), so z1 collapses to a single K=68 matmul
against [rb(32); onehot_ptype(20); onehot_ltype(16)] with a fused weight
[W1c; prot_emb@W1a; lig_emb@W1b + b1].

The ~230 tiles of all 8 complexes are bin-packed across the 8 cores (29 per
core); per-tile channel sums (acc) are attributed back to complexes on the
host.  Device work per tile: 3 fp16 matmuls (K=68 z1, W2, W3; N=512, 1
col/cycle) + 3 relu evacuations split DVE/ACT, pair-sums fused into the
accum_out port.  A tiny N=256 filler matmul per step keeps the PE's HAM
clock-gate at 2.4 GHz (micro-idle gaps throttle it to 1.2 GHz).

Pad columns (zero rhs) contribute relu(W3.T relu(b2) + b3); that constant
is extracted from a guaranteed-pad column (cvec) and the host subtracts
npad*cvec per complex, then runs the tiny fp32 scoring head while
unsharding.
"""
import numpy as np
from contextlib import ExitStack

import concourse.bass as bass
import concourse.bacc as bacc
import concourse.tile as tile
from concourse import mybir
from concourse import bass_utils

F32 = mybir.dt.float32
FP16 = mybir.dt.float16
AF = mybir.ActivationFunctionType
ALU = mybir.AluOpType

B, P, L = 8, 512, 64
H, RB = 128, 32
NPT, NLT = 20, 16
CUTOFF = 8.0
N_CORES = 8
K1 = RB + NPT + NLT          # 68: contraction dim of the fused z1 matmul
NT_DEFAULT = 29              # tiles of 512 pairs per core
WIDTH = 0.5 * CUTOFF / RB + 1e-8

_CACHE = {}


def _chunk_layout(nt):
    """Chunks of tiles per dma_start: a 1-tile first chunk so the pipeline
    starts fast, then 4-tile chunks."""
    sizes = [1]
    while sum(sizes) < nt:
        sizes.append(min(4, nt - sum(sizes)))
    starts = np.cumsum([0] + sizes).tolist()
    tile2chunk = {}
    for c, (st, sz) in enumerate(zip(starts, sizes)):
        for s in range(sz):
            tile2chunk[st + s] = (c, s)
    return sizes, starts, tile2chunk


def _build_nc(nt):
    nc = bacc.Bacc("TRN2", target_bir_lowering=False, debug=False,
                   num_devices=N_CORES)
    d = {}

    def inp(name, shape, dt):
        d[name] = nc.dram_tensor(name, shape, dt, kind="ExternalInput").ap()

    inp("rhsG", [K1, nt * 512], FP16)
    inp("wa", [K1, H], FP16)      # W1z
    inp("wb", [H, 2 * H], FP16)   # W2 | W3
    inp("fb", [H, 2], F32)        # b2 | b3

    acc_ap = nc.dram_tensor("acc", [H, nt], F32, kind="ExternalOutput").ap()
    cvec_ap = nc.dram_tensor("cvec", [H, 1], F32, kind="ExternalOutput").ap()

    sizes, starts, tile2chunk = _chunk_layout(nt)
    nchunks = len(sizes)

    with tile.TileContext(nc) as tc:
        with ExitStack() as ctx:
            const = ctx.enter_context(tc.tile_pool(name="const", bufs=1))
            rbuf = ctx.enter_context(tc.tile_pool(name="rbuf", bufs=3))
            abuf = ctx.enter_context(tc.tile_pool(name="abuf", bufs=3))
            psZ1 = ctx.enter_context(tc.tile_pool(name="psZ1", bufs=3, space="PSUM"))
            psZ2 = ctx.enter_context(tc.tile_pool(name="psZ2", bufs=2, space="PSUM"))
            psZ3 = ctx.enter_context(tc.tile_pool(name="psZ3", bufs=2, space="PSUM"))
            psF = ctx.enter_context(tc.tile_pool(name="psF", bufs=1, space="PSUM"))

            chunks, z1s, a1s, z2s, a2s, z3s = {}, {}, {}, {}, {}, {}

            def dma_chunk(c):
                lo, w = starts[c] * 512, sizes[c] * 512
                rt = rbuf.tile([K1, 4 * 512], FP16, tag="rhs", name=f"rhs{c}")
                nc.sync.dma_start(out=rt[:, 0:w], in_=d["rhsG"][:, lo:lo + w])
                chunks[c] = rt

            # first-tile weights + 1-tile chunk land first so MM0 starts ASAP
            wa = const.tile([K1, H], FP16, tag="wa", name="wa")
            nc.sync.dma_start(out=wa, in_=d["wa"])
            dma_chunk(0)
            wb = const.tile([H, 2 * H], FP16, tag="wb", name="wb")
            nc.sync.dma_start(out=wb, in_=d["wb"])
            fb = const.tile([H, 2], F32, tag="fb", name="fb")
            nc.sync.dma_start(out=fb, in_=d["fb"])
            dma_chunk(1)
            dma_chunk(2)
            W1z = wa[:, :]
            W2 = wb[:, 0:H]
            W3 = wb[:, H:2 * H]

            # warm the ACT table set before the pipeline needs relu
            warm = const.tile([1, 64], F32, tag="warm", name="warm")
            nc.vector.memset(warm[:, :], 0.0)
            nc.scalar.activation(out=warm[:, :], in_=warm[:, :], func=AF.Relu,
                                 bias=0.0, scale=1.0)
            # biases as dense [H,1] tiles
            b2 = const.tile([H, 1], F32, tag="b2", name="b2")
            nc.vector.tensor_scalar(out=b2[:, :], in0=fb[:, 0:1],
                                    scalar1=0.0, scalar2=None, op0=ALU.add)
            b3 = const.tile([H, 1], F32, tag="b3", name="b3")
            nc.vector.tensor_scalar(out=b3[:, :], in0=fb[:, 1:2],
                                    scalar1=0.0, scalar2=None, op0=ALU.add)

            acc = const.tile([H, nt], F32, tag="acc", name="acc")
            a3_last = const.tile([H, 512], F32, tag="a3_last", name="a3_last")
            zeros = const.tile([H, 512], F32, tag="zeros", name="zeros")
            nc.vector.memset(zeros[:, :], 0.0)
            # dependency-free rhs for the HAM-warming filler matmuls
            fscr = const.tile([K1, 256], FP16, tag="fscr", name="fscr")
            nc.vector.memset(fscr[:, :], 0.0)
            fill_ps = psF.tile([H, 256], F32, tag="fill", name="fill_ps")

            # relu3 engine split: DVE for the tail (pad tiles must match the
            # extracted cvec exactly) + 1/3 of the rest to balance engines
            def r3_on_dve(tt):
                return (tt >= nt - 8) or (tt % 3 == 0)

            for step in range(nt + 6):
                t0 = step
                if t0 < nt:
                    c0, s0 = tile2chunk[t0]
                    if s0 == 0 and c0 + 3 < nchunks:
                        dma_chunk(c0 + 3)
                    z1 = psZ1.tile([H, 512], F32, tag="z1", name=f"z1_{t0}")
                    z1s[t0] = z1
                    if step > 0:
                        nc.tensor.matmul(out=fill_ps[:, :],
                                         lhsT=W1z, rhs=fscr[:, :],
                                         start=True, stop=True)
                    nc.tensor.matmul(out=z1[:, :], lhsT=W1z,
                                     rhs=chunks[c0][:, 512 * s0:512 * (s0 + 1)],
                                     start=True, stop=True)
                t1 = step - 1
                if 0 <= t1 < nt:
                    a1 = abuf.tile([H, 512], FP16, tag="a1", name=f"a1_{t1}")
                    a1s[t1] = a1
                    nc.vector.tensor_scalar(out=a1[:, :],
                                            in0=z1s.pop(t1)[:, :],
                                            scalar1=0.0, scalar2=None,
                                            op0=ALU.max)
                t2 = step - 2
                if 0 <= t2 < nt:
                    z2 = psZ2.tile([H, 512], F32, tag="z2", name=f"z2_{t2}")
                    z2s[t2] = z2
                    nc.tensor.matmul(out=z2[:, :], lhsT=W2,
                                     rhs=a1s.pop(t2)[:, :],
                                     start=True, stop=True)
                t3 = step - 3
                if 0 <= t3 < nt:
                    a2 = abuf.tile([H, 512], FP16, tag="a2", name=f"a2_{t3}")
                    a2s[t3] = a2
                    nc.scalar.activation(out=a2[:, :], in_=z2s.pop(t3)[:, :],
                                         func=AF.Relu, bias=b2, scale=1.0)
                t4 = step - 4
                if 0 <= t4 < nt:
                    z3 = psZ3.tile([H, 512], F32, tag="z3", name=f"z3_{t4}")
                    z3s[t4] = z3
                    nc.tensor.matmul(out=z3[:, :], lhsT=W3,
                                     rhs=a2s.pop(t4)[:, :],
                                     start=True, stop=True)
                t5 = step - 5
                if 0 <= t5 < nt:
                    if t5 == nt - 1:
                        a3 = a3_last
                    else:
                        a3 = abuf.tile([H, 512], FP16, tag="a3",
                                       name=f"a3_{t5}")
                    z3ap = z3s.pop(t5)
                    if r3_on_dve(t5):
                        # out = max(z3 + b3, 0); accum = sum(out)
                        nc.vector.scalar_tensor_tensor(
                            out=a3[:, :], in0=z3ap[:, :],
                            scalar=b3, in1=zeros[:, :],
                            op0=ALU.add, op1=ALU.max,
                            accum_out=acc[:, t5:t5 + 1])
                    else:
                        nc.scalar.activation(out=a3[:, :], in_=z3ap[:, :],
                                             func=AF.Relu, bias=b3,
                                             scale=1.0,
                                             accum_out=acc[:, t5:t5 + 1])

            nc.sync.dma_start(out=acc_ap, in_=acc[:, :])
            nc.sync.dma_start(out=cvec_ap, in_=a3_last[:, 511:512])

    nc.compile()
    return nc


def _get_nc(nt=NT_DEFAULT):
    if nt not in _CACHE:
        _CACHE[nt] = _build_nc(nt)
    return _CACHE[nt]


def kernel(protein_pos, ligand_pos, prot_emb, lig_emb,
           W1, b1, W2, b2, W3, b3, Wr1, br1, Wr2, br2,
           protein_atom_type, ligand_atom_type, protein_batch, ligand_batch):
    protein_pos = np.asarray(protein_pos, dtype=np.float32).reshape(B, P, 3)
    ligand_pos = np.asarray(ligand_pos, dtype=np.float32).reshape(B, L, 3)
    prot_emb = np.asarray(prot_emb, dtype=np.float32)
    lig_emb = np.asarray(lig_emb, dtype=np.float32)
    W1 = np.asarray(W1, dtype=np.float32)
    b1 = np.asarray(b1, dtype=np.float32)
    W2 = np.asarray(W2, dtype=np.float32)
    W3 = np.asarray(W3, dtype=np.float32)
    Wr1 = np.asarray(Wr1, dtype=np.float32)
    br1 = np.asarray(br1, dtype=np.float32).reshape(H)
    Wr2 = np.asarray(Wr2, dtype=np.float32).reshape(H)
    br2 = float(np.asarray(br2, dtype=np.float32).reshape(()))
    ptype = np.asarray(protein_atom_type).reshape(B, P)
    ltype = np.asarray(ligand_atom_type).reshape(B, L)

    # fused stage-1 weight: z1 = W1z.T @ [rb; onehot_p; onehot_l]
    PA = prot_emb @ W1[0:H, :]
    LA = lig_emb @ W1[H:2 * H, :] + b1[None, :]
    W1z = np.concatenate([W1[2 * H:2 * H + RB, :], PA, LA],
                         axis=0).astype(np.float16)
    wbb = np.concatenate([W2, W3], axis=1).astype(np.float16)
    fbb = np.ascontiguousarray(
        np.stack([b2.reshape(H), b3.reshape(H)], axis=1).astype(np.float32))

    centers = np.linspace(0.0, CUTOFF, RB, dtype=np.float32)

    # per-complex valid pairs -> global list of 512-pair tiles
    full_tiles = []     # (complex, rhs [K1,512] fp32) with no pad columns
    padded_tiles = []   # each complex's final, partially-padded tile
    cnts = []
    for b in range(B):
        diff = protein_pos[b][:, None, :] - ligand_pos[b][None, :, :]
        dist = np.sqrt((diff * diff).sum(-1, dtype=np.float32))
        pi, li = np.nonzero(dist < np.float32(CUTOFF))
        cnt = len(pi)
        cnts.append(cnt)
        dv = dist[pi, li]
        rhs = np.zeros((K1, ((cnt + 511) // 512) * 512), dtype=np.float32)
        rhs[0:RB, :cnt] = np.exp(
            -0.5 * ((dv[:, None] - centers[None, :]) / WIDTH) ** 2).T
        rhs[RB + ptype[b][pi], np.arange(cnt)] = 1.0
        rhs[RB + NPT + ltype[b][li], np.arange(cnt)] = 1.0
        nt_b = rhs.shape[1] // 512
        for s in range(nt_b):
            t = (b, rhs[:, 512 * s:512 * (s + 1)])
            if s == nt_b - 1 and cnt % 512 != 0:
                padded_tiles.append(t)
            else:
                full_tiles.append(t)

    ntot = len(full_tiles) + len(padded_tiles)
    nt = max(NT_DEFAULT, (ntot + N_CORES - 1) // N_CORES)
    # every core's LAST slot must contain a pad at column 511 (cvec source):
    # full tiles round-robin first, then one padded tile per core at the end;
    # any remaining slots become all-pad filler tiles.
    while True:
        core_tiles = [[] for _ in range(N_CORES)]
        core_pad = [False] * N_CORES
        for i, tb in enumerate(full_tiles):
            core_tiles[i % N_CORES].append(tb)
        for i, tb in enumerate(padded_tiles):
            core_tiles[i % N_CORES].append(tb)
            core_pad[i % N_CORES] = True
        ok = all(len(ct) <= nt and (len(ct) < nt or core_pad[k])
                 for k, ct in enumerate(core_tiles))
        if ok:
            break
        nt += 1
    npair = nt * 512

    in_maps = []
    tilemap = []                 # per core: complex id per tile (-1 = pad)
    for k in range(N_CORES):
        rhs = np.zeros((K1, npair), dtype=np.float32)
        cmap = []
        for s, (b, rt) in enumerate(core_tiles[k]):
            rhs[:, 512 * s:512 * (s + 1)] = rt
            cmap.append(b)
        cmap += [-1] * (nt - len(cmap))
        tilemap.append(cmap)
        in_maps.append({"rhsG": rhs.astype(np.float16), "wa": W1z,
                        "wb": wbb, "fb": fbb})

    nc = _get_nc(nt)
    res = bass_utils.run_bass_kernel_spmd(nc, in_maps,
                                          core_ids=list(range(N_CORES)))

    # unshard: attribute tile sums to complexes, remove pad contributions,
    # mean + tiny fp32 scoring head
    cvec = res.results[0]["cvec"][:, 0]
    tot = np.zeros((B, H), dtype=np.float64)
    ntiles_b = np.zeros(B, dtype=np.int64)
    for k in range(N_CORES):
        acc = res.results[k]["acc"]
        for s, b in enumerate(tilemap[k]):
            if b >= 0:
                tot[b] += acc[:, s]
                ntiles_b[b] += 1
    out = np.zeros(B, dtype=np.float32)
    for b in range(B):
        cnt = cnts[b]
        if cnt == 0:
            continue
        npad = ntiles_b[b] * 512 - cnt
        t2 = (tot[b] - npad * cvec).astype(np.float32)
        repr_ = (t2 / cnt).astype(np.float32)
        r1 = np.maximum(repr_ @ Wr1 + br1, 0.0)
        out[b] = r1 @ Wr2 + br2
    return out


# revision 17
# speedup vs baseline: 1.3444x; 1.2266x over previous
"""DockingScorePredictor Trainium2 kernel.

Only pairs inside the 8A cutoff matter (~43%), so the host compacts valid
pairs into dense 512-pair tiles.  The per-pair layer-1 preactivation
  z1 = W1a.T hp + W1b.T hl + b1 + W1c.T rb(d)
depends on atom *types* (20/16), so z1 collapses to a single K=68 matmul
against [rb(32); onehot_ptype(20); onehot_ltype(16)] with a fused weight
[W1c; prot_emb@W1a; lig_emb@W1b + b1].

The ~230 tiles of all 8 complexes are bin-packed across the 8 cores (29 per
core); per-tile channel sums (acc) are attributed back to complexes on the
host.  Device work per tile: 3 fp16 matmuls (K=68 z1, W2, W3; N=512, 1
col/cycle) + 3 relu evacuations split DVE/ACT, pair-sums fused into the
accum_out port.  A tiny N=256 filler matmul per step keeps the PE's HAM
clock-gate at 2.4 GHz (micro-idle gaps throttle it to 1.2 GHz).

Pad columns (zero rhs) contribute relu(W3.T relu(b2) + b3); that constant
is extracted from a guaranteed-pad column (cvec) and the host subtracts
npad*cvec per complex, then runs the tiny fp32 scoring head while
unsharding.
"""
import numpy as np
from contextlib import ExitStack

import concourse.bass as bass
import concourse.bacc as bacc
import concourse.tile as tile
from concourse import mybir
from concourse import bass_utils

F32 = mybir.dt.float32
FP16 = mybir.dt.float16
AF = mybir.ActivationFunctionType
ALU = mybir.AluOpType

B, P, L = 8, 512, 64
H, RB = 128, 32
NPT, NLT = 20, 16
CUTOFF = 8.0
N_CORES = 8
K1 = RB + NPT + NLT          # 68: contraction dim of the fused z1 matmul
NT_DEFAULT = 29              # tiles of 512 pairs per core
WIDTH = 0.5 * CUTOFF / RB + 1e-8

_CACHE = {}


def _build_nc(nt):
    nc = bacc.Bacc("TRN2", target_bir_lowering=False, debug=False,
                   num_devices=N_CORES)
    d = {}

    def inp(name, shape, dt):
        d[name] = nc.dram_tensor(name, shape, dt, kind="ExternalInput").ap()

    inp("rhsG", [K1, nt * 512], FP16)
    inp("wa", [K1, H], FP16)      # W1z
    inp("wb", [H, 2 * H], FP16)   # W2 | W3
    inp("fb", [H, 2], F32)        # b2 | b3

    acc_ap = nc.dram_tensor("acc", [H, nt], F32, kind="ExternalOutput").ap()
    cvec_ap = nc.dram_tensor("cvec", [H, 1], F32, kind="ExternalOutput").ap()

    with tile.TileContext(nc) as tc:
        with ExitStack() as ctx:
            const = ctx.enter_context(tc.tile_pool(name="const", bufs=1))
            rbuf = ctx.enter_context(tc.tile_pool(name="rbuf", bufs=6))
            abuf = ctx.enter_context(tc.tile_pool(name="abuf", bufs=3))
            psZ1 = ctx.enter_context(tc.tile_pool(name="psZ1", bufs=3, space="PSUM"))
            psZ2 = ctx.enter_context(tc.tile_pool(name="psZ2", bufs=3, space="PSUM"))
            psZ3 = ctx.enter_context(tc.tile_pool(name="psZ3", bufs=2, space="PSUM"))

            rtiles, z1s, a1s, z2s, a2s, z3s = {}, {}, {}, {}, {}, {}
            PREF = 5

            def dma_rhs(g):
                rt = rbuf.tile([K1, 512], FP16, tag="rhs", name=f"rhs{g}")
                nc.sync.dma_start(out=rt[:, :],
                                  in_=d["rhsG"][:, 512 * g:512 * (g + 1)])
                rtiles[g] = rt

            # first-tile weights + first rhs tile land first so MM0 starts ASAP
            wa = const.tile([K1, H], FP16, tag="wa", name="wa")
            nc.sync.dma_start(out=wa, in_=d["wa"])
            dma_rhs(0)
            wb = const.tile([H, 2 * H], FP16, tag="wb", name="wb")
            nc.sync.dma_start(out=wb, in_=d["wb"])
            fb = const.tile([H, 2], F32, tag="fb", name="fb")
            nc.sync.dma_start(out=fb, in_=d["fb"])
            for g in range(1, PREF):
                dma_rhs(g)
            W1z = wa[:, :]
            W2 = wb[:, 0:H]
            W3 = wb[:, H:2 * H]

            # warm the ACT table set before the pipeline needs relu
            warm = const.tile([1, 64], F32, tag="warm", name="warm")
            nc.vector.memset(warm[:, :], 0.0)
            nc.scalar.activation(out=warm[:, :], in_=warm[:, :], func=AF.Relu,
                                 bias=0.0, scale=1.0)
            # biases as dense [H,1] tiles
            b2 = const.tile([H, 1], F32, tag="b2", name="b2")
            nc.vector.tensor_scalar(out=b2[:, :], in0=fb[:, 0:1],
                                    scalar1=0.0, scalar2=None, op0=ALU.add)
            b3 = const.tile([H, 1], F32, tag="b3", name="b3")
            nc.vector.tensor_scalar(out=b3[:, :], in0=fb[:, 1:2],
                                    scalar1=0.0, scalar2=None, op0=ALU.add)

            acc = const.tile([H, nt], F32, tag="acc", name="acc")
            a3_last = const.tile([H, 512], F32, tag="a3_last", name="a3_last")
            zeros = const.tile([H, 512], F32, tag="zeros", name="zeros")
            nc.vector.memset(zeros[:, :], 0.0)
            # relu3 engine split: DVE for the tail (pad tiles must match the
            # extracted cvec exactly) + 1/4 of the rest to balance engines
            def r3_on_dve(tt):
                return (tt >= nt - 8) or (tt % 8 < 2)

            for step in range(nt + 6):
                t0 = step
                if t0 < nt:
                    if t0 + PREF < nt:
                        dma_rhs(t0 + PREF)
                    z1 = psZ1.tile([H, 512], F32, tag="z1", name=f"z1_{t0}")
                    z1s[t0] = z1
                    nc.tensor.matmul(out=z1[:, :], lhsT=W1z,
                                     rhs=rtiles.pop(t0)[:, :],
                                     start=True, stop=True)
                t1 = step - 1
                if 0 <= t1 < nt:
                    a1 = abuf.tile([H, 512], FP16, tag="a1", name=f"a1_{t1}")
                    a1s[t1] = a1
                    nc.vector.tensor_scalar(out=a1[:, :],
                                            in0=z1s.pop(t1)[:, :],
                                            scalar1=0.0, scalar2=None,
                                            op0=ALU.max)
                t2 = step - 2
                if 0 <= t2 < nt:
                    z2 = psZ2.tile([H, 512], F32, tag="z2", name=f"z2_{t2}")
                    z2s[t2] = z2
                    nc.tensor.matmul(out=z2[:, :], lhsT=W2,
                                     rhs=a1s.pop(t2)[:, :],
                                     start=True, stop=True)
                t3 = step - 3
                if 0 <= t3 < nt:
                    a2 = abuf.tile([H, 512], FP16, tag="a2", name=f"a2_{t3}")
                    a2s[t3] = a2
                    nc.scalar.activation(out=a2[:, :], in_=z2s.pop(t3)[:, :],
                                         func=AF.Relu, bias=b2, scale=1.0)
                t4 = step - 4
                if 0 <= t4 < nt:
                    z3 = psZ3.tile([H, 512], F32, tag="z3", name=f"z3_{t4}")
                    z3s[t4] = z3
                    nc.tensor.matmul(out=z3[:, :], lhsT=W3,
                                     rhs=a2s.pop(t4)[:, :],
                                     start=True, stop=True)
                t5 = step - 5
                if 0 <= t5 < nt:
                    if t5 == nt - 1:
                        a3 = a3_last
                    else:
                        a3 = abuf.tile([H, 512], FP16, tag="a3",
                                       name=f"a3_{t5}")
                    z3ap = z3s.pop(t5)
                    if r3_on_dve(t5):
                        # out = max(z3 + b3, 0); accum = sum(out)
                        nc.vector.scalar_tensor_tensor(
                            out=a3[:, :], in0=z3ap[:, :],
                            scalar=b3, in1=zeros[:, :],
                            op0=ALU.add, op1=ALU.max,
                            accum_out=acc[:, t5:t5 + 1])
                    else:
                        nc.scalar.activation(out=a3[:, :], in_=z3ap[:, :],
                                             func=AF.Relu, bias=b3,
                                             scale=1.0,
                                             accum_out=acc[:, t5:t5 + 1])

            nc.sync.dma_start(out=acc_ap, in_=acc[:, :])
            nc.sync.dma_start(out=cvec_ap, in_=a3_last[:, 511:512])

    nc.compile()
    return nc


def _get_nc(nt=NT_DEFAULT):
    if nt not in _CACHE:
        _CACHE[nt] = _build_nc(nt)
    return _CACHE[nt]


def kernel(protein_pos, ligand_pos, prot_emb, lig_emb,
           W1, b1, W2, b2, W3, b3, Wr1, br1, Wr2, br2,
           protein_atom_type, ligand_atom_type, protein_batch, ligand_batch):
    protein_pos = np.asarray(protein_pos, dtype=np.float32).reshape(B, P, 3)
    ligand_pos = np.asarray(ligand_pos, dtype=np.float32).reshape(B, L, 3)
    prot_emb = np.asarray(prot_emb, dtype=np.float32)
    lig_emb = np.asarray(lig_emb, dtype=np.float32)
    W1 = np.asarray(W1, dtype=np.float32)
    b1 = np.asarray(b1, dtype=np.float32)
    W2 = np.asarray(W2, dtype=np.float32)
    W3 = np.asarray(W3, dtype=np.float32)
    Wr1 = np.asarray(Wr1, dtype=np.float32)
    br1 = np.asarray(br1, dtype=np.float32).reshape(H)
    Wr2 = np.asarray(Wr2, dtype=np.float32).reshape(H)
    br2 = float(np.asarray(br2, dtype=np.float32).reshape(()))
    ptype = np.asarray(protein_atom_type).reshape(B, P)
    ltype = np.asarray(ligand_atom_type).reshape(B, L)

    # fused stage-1 weight: z1 = W1z.T @ [rb; onehot_p; onehot_l]
    PA = prot_emb @ W1[0:H, :]
    LA = lig_emb @ W1[H:2 * H, :] + b1[None, :]
    W1z = np.concatenate([W1[2 * H:2 * H + RB, :], PA, LA],
                         axis=0).astype(np.float16)
    wbb = np.concatenate([W2, W3], axis=1).astype(np.float16)
    fbb = np.ascontiguousarray(
        np.stack([b2.reshape(H), b3.reshape(H)], axis=1).astype(np.float32))

    centers = np.linspace(0.0, CUTOFF, RB, dtype=np.float32)

    # per-complex valid pairs -> global list of 512-pair tiles
    full_tiles = []     # (complex, rhs [K1,512] fp32) with no pad columns
    padded_tiles = []   # each complex's final, partially-padded tile
    cnts = []
    for b in range(B):
        diff = protein_pos[b][:, None, :] - ligand_pos[b][None, :, :]
        dist = np.sqrt((diff * diff).sum(-1, dtype=np.float32))
        pi, li = np.nonzero(dist < np.float32(CUTOFF))
        cnt = len(pi)
        cnts.append(cnt)
        dv = dist[pi, li]
        rhs = np.zeros((K1, ((cnt + 511) // 512) * 512), dtype=np.float32)
        rhs[0:RB, :cnt] = np.exp(
            -0.5 * ((dv[:, None] - centers[None, :]) / WIDTH) ** 2).T
        rhs[RB + ptype[b][pi], np.arange(cnt)] = 1.0
        rhs[RB + NPT + ltype[b][li], np.arange(cnt)] = 1.0
        nt_b = rhs.shape[1] // 512
        for s in range(nt_b):
            t = (b, rhs[:, 512 * s:512 * (s + 1)])
            if s == nt_b - 1 and cnt % 512 != 0:
                padded_tiles.append(t)
            else:
                full_tiles.append(t)

    ntot = len(full_tiles) + len(padded_tiles)
    nt = max(NT_DEFAULT, (ntot + N_CORES - 1) // N_CORES)
    # every core's LAST slot must contain a pad at column 511 (cvec source):
    # full tiles round-robin first, then one padded tile per core at the end;
    # any remaining slots become all-pad filler tiles.
    while True:
        core_tiles = [[] for _ in range(N_CORES)]
        core_pad = [False] * N_CORES
        for i, tb in enumerate(full_tiles):
            core_tiles[i % N_CORES].append(tb)
        for i, tb in enumerate(padded_tiles):
            core_tiles[i % N_CORES].append(tb)
            core_pad[i % N_CORES] = True
        ok = all(len(ct) <= nt and (len(ct) < nt or core_pad[k])
                 for k, ct in enumerate(core_tiles))
        if ok:
            break
        nt += 1
    npair = nt * 512

    in_maps = []
    tilemap = []                 # per core: complex id per tile (-1 = pad)
    for k in range(N_CORES):
        rhs = np.zeros((K1, npair), dtype=np.float32)
        cmap = []
        for s, (b, rt) in enumerate(core_tiles[k]):
            rhs[:, 512 * s:512 * (s + 1)] = rt
            cmap.append(b)
        cmap += [-1] * (nt - len(cmap))
        tilemap.append(cmap)
        in_maps.append({"rhsG": rhs.astype(np.float16), "wa": W1z,
                        "wb": wbb, "fb": fbb})

    nc = _get_nc(nt)
    res = bass_utils.run_bass_kernel_spmd(nc, in_maps,
                                          core_ids=list(range(N_CORES)))

    # unshard: attribute tile sums to complexes, remove pad contributions,
    # mean + tiny fp32 scoring head
    cvec = res.results[0]["cvec"][:, 0]
    tot = np.zeros((B, H), dtype=np.float64)
    ntiles_b = np.zeros(B, dtype=np.int64)
    for k in range(N_CORES):
        acc = res.results[k]["acc"]
        for s, b in enumerate(tilemap[k]):
            if b >= 0:
                tot[b] += acc[:, s]
                ntiles_b[b] += 1
    out = np.zeros(B, dtype=np.float32)
    for b in range(B):
        cnt = cnts[b]
        if cnt == 0:
            continue
        npad = ntiles_b[b] * 512 - cnt
        t2 = (tot[b] - npad * cvec).astype(np.float32)
        repr_ = (t2 / cnt).astype(np.float32)
        r1 = np.maximum(repr_ @ Wr1 + br1, 0.0)
        out[b] = r1 @ Wr2 + br2
    return out


# revision 18
# speedup vs baseline: 1.4126x; 1.0507x over previous
"""DockingScorePredictor Trainium2 kernel.

Only pairs inside the 8A cutoff matter (~43%), so the host compacts valid
pairs into dense 512-pair tiles.  The per-pair layer-1 preactivation
  z1 = W1a.T hp + W1b.T hl + b1 + W1c.T rb(d)
depends on atom *types* (20/16), so z1 collapses to a single K=68 matmul
against [rb(32); onehot_ptype(20); onehot_ltype(16)] with a fused weight
[W1c; prot_emb@W1a; lig_emb@W1b + b1].

The ~230 tiles of all 8 complexes are bin-packed across the 8 cores (29 per
core); per-tile channel sums (acc) are attributed back to complexes on the
host.  Device work per tile: 3 fp16 matmuls (K=68 z1, W2, W3; N=512, 1
col/cycle) + 3 relu evacuations split DVE/ACT, pair-sums fused into the
accum_out port.  A tiny N=256 filler matmul per step keeps the PE's HAM
clock-gate at 2.4 GHz (micro-idle gaps throttle it to 1.2 GHz).

Pad columns (zero rhs) contribute relu(W3.T relu(b2) + b3); that constant
is extracted from a guaranteed-pad column (cvec) and the host subtracts
npad*cvec per complex, then runs the tiny fp32 scoring head while
unsharding.
"""
import numpy as np
from contextlib import ExitStack

import concourse.bass as bass
import concourse.bacc as bacc
import concourse.tile as tile
from concourse import mybir
from concourse import bass_utils

F32 = mybir.dt.float32
FP16 = mybir.dt.float16
AF = mybir.ActivationFunctionType
ALU = mybir.AluOpType

B, P, L = 8, 512, 64
H, RB = 128, 32
NPT, NLT = 20, 16
CUTOFF = 8.0
N_CORES = 8
K1 = RB + NPT + NLT          # 68: contraction dim of the fused z1 matmul
NT_DEFAULT = 29              # tiles of 512 pairs per core
WIDTH = 0.5 * CUTOFF / RB + 1e-8

_CACHE = {}


def _build_nc(nt):
    nc = bacc.Bacc("TRN2", target_bir_lowering=False, debug=False,
                   num_devices=N_CORES)
    d = {}

    def inp(name, shape, dt):
        d[name] = nc.dram_tensor(name, shape, dt, kind="ExternalInput").ap()

    inp("rhsG", [K1, nt * 512], FP16)
    inp("wa", [K1, H], FP16)      # W1z
    inp("wb", [H, 2 * H], FP16)   # W2 | W3
    inp("fb", [H, 2], F32)        # b2 | b3

    acc_ap = nc.dram_tensor("acc", [H, nt], F32, kind="ExternalOutput").ap()
    cvec_ap = nc.dram_tensor("cvec", [H, 1], F32, kind="ExternalOutput").ap()

    with tile.TileContext(nc) as tc:
        with ExitStack() as ctx:
            const = ctx.enter_context(tc.tile_pool(name="const", bufs=1))
            rbuf = ctx.enter_context(tc.tile_pool(name="rbuf", bufs=6))
            abuf = ctx.enter_context(tc.tile_pool(name="abuf", bufs=3))
            psZ1 = ctx.enter_context(tc.tile_pool(name="psZ1", bufs=3, space="PSUM"))
            psZ2 = ctx.enter_context(tc.tile_pool(name="psZ2", bufs=3, space="PSUM"))
            psZ3 = ctx.enter_context(tc.tile_pool(name="psZ3", bufs=2, space="PSUM"))

            rtiles, z1s, a1s, z2s, a2s, z3s = {}, {}, {}, {}, {}, {}
            PREF = 5

            def dma_rhs(g):
                rt = rbuf.tile([K1, 512], FP16, tag="rhs", name=f"rhs{g}")
                nc.sync.dma_start(out=rt[:, :],
                                  in_=d["rhsG"][:, 512 * g:512 * (g + 1)])
                rtiles[g] = rt

            # first-tile weights + first rhs tile land first so MM0 starts ASAP
            wa = const.tile([K1, H], FP16, tag="wa", name="wa")
            nc.sync.dma_start(out=wa, in_=d["wa"])
            dma_rhs(0)
            wb = const.tile([H, 2 * H], FP16, tag="wb", name="wb")
            nc.sync.dma_start(out=wb, in_=d["wb"])
            fb = const.tile([H, 2], F32, tag="fb", name="fb")
            nc.sync.dma_start(out=fb, in_=d["fb"])
            for g in range(1, PREF):
                dma_rhs(g)
            W1z = wa[:, :]
            W2 = wb[:, 0:H]
            W3 = wb[:, H:2 * H]

            # warm the ACT table set before the pipeline needs relu
            warm = const.tile([1, 64], F32, tag="warm", name="warm")
            nc.vector.memset(warm[:, :], 0.0)
            nc.scalar.activation(out=warm[:, :], in_=warm[:, :], func=AF.Relu,
                                 bias=0.0, scale=1.0)
            # biases as dense [H,1] tiles
            b2 = const.tile([H, 1], F32, tag="b2", name="b2")
            nc.vector.tensor_scalar(out=b2[:, :], in0=fb[:, 0:1],
                                    scalar1=0.0, scalar2=None, op0=ALU.add)
            b3 = const.tile([H, 1], F32, tag="b3", name="b3")
            nc.vector.tensor_scalar(out=b3[:, :], in0=fb[:, 1:2],
                                    scalar1=0.0, scalar2=None, op0=ALU.add)

            acc = const.tile([H, nt], F32, tag="acc", name="acc")
            a3_last = const.tile([H, 512], F32, tag="a3_last", name="a3_last")
            zeros = const.tile([H, 512], F32, tag="zeros", name="zeros")
            nc.vector.memset(zeros[:, :], 0.0)
            # relu3 engine split: DVE for the tail (pad tiles must match the
            # extracted cvec exactly) + half of the rest; the DVE-heavy split
            # makes DVE the single pacer, which schedules with fewer bubbles
            # than a perfectly balanced DVE/ACT assignment
            def r3_on_dve(tt):
                return (tt >= nt - 8) or (tt % 8 < 4)

            for step in range(nt + 6):
                t0 = step
                if t0 < nt:
                    if t0 + PREF < nt:
                        dma_rhs(t0 + PREF)
                    z1 = psZ1.tile([H, 512], F32, tag="z1", name=f"z1_{t0}")
                    z1s[t0] = z1
                    nc.tensor.matmul(out=z1[:, :], lhsT=W1z,
                                     rhs=rtiles.pop(t0)[:, :],
                                     start=True, stop=True)
                t1 = step - 1
                if 0 <= t1 < nt:
                    a1 = abuf.tile([H, 512], FP16, tag="a1", name=f"a1_{t1}")
                    a1s[t1] = a1
                    nc.vector.tensor_scalar(out=a1[:, :],
                                            in0=z1s.pop(t1)[:, :],
                                            scalar1=0.0, scalar2=None,
                                            op0=ALU.max)
                t2 = step - 2
                if 0 <= t2 < nt:
                    z2 = psZ2.tile([H, 512], F32, tag="z2", name=f"z2_{t2}")
                    z2s[t2] = z2
                    nc.tensor.matmul(out=z2[:, :], lhsT=W2,
                                     rhs=a1s.pop(t2)[:, :],
                                     start=True, stop=True)
                t3 = step - 3
                if 0 <= t3 < nt:
                    a2 = abuf.tile([H, 512], FP16, tag="a2", name=f"a2_{t3}")
                    a2s[t3] = a2
                    nc.scalar.activation(out=a2[:, :], in_=z2s.pop(t3)[:, :],
                                         func=AF.Relu, bias=b2, scale=1.0)
                t4 = step - 4
                if 0 <= t4 < nt:
                    z3 = psZ3.tile([H, 512], F32, tag="z3", name=f"z3_{t4}")
                    z3s[t4] = z3
                    nc.tensor.matmul(out=z3[:, :], lhsT=W3,
                                     rhs=a2s.pop(t4)[:, :],
                                     start=True, stop=True)
                t5 = step - 5
                if 0 <= t5 < nt:
                    if t5 == nt - 1:
                        a3 = a3_last
                    else:
                        a3 = abuf.tile([H, 512], FP16, tag="a3",
                                       name=f"a3_{t5}")
                    z3ap = z3s.pop(t5)
                    if r3_on_dve(t5):
                        # out = max(z3 + b3, 0); accum = sum(out)
                        nc.vector.scalar_tensor_tensor(
                            out=a3[:, :], in0=z3ap[:, :],
                            scalar=b3, in1=zeros[:, :],
                            op0=ALU.add, op1=ALU.max,
                            accum_out=acc[:, t5:t5 + 1])
                    else:
                        nc.scalar.activation(out=a3[:, :], in_=z3ap[:, :],
                                             func=AF.Relu, bias=b3,
                                             scale=1.0,
                                             accum_out=acc[:, t5:t5 + 1])

            nc.sync.dma_start(out=acc_ap, in_=acc[:, :])
            nc.sync.dma_start(out=cvec_ap, in_=a3_last[:, 511:512])

    nc.compile()
    return nc


def _get_nc(nt=NT_DEFAULT):
    if nt not in _CACHE:
        _CACHE[nt] = _build_nc(nt)
    return _CACHE[nt]


def kernel(protein_pos, ligand_pos, prot_emb, lig_emb,
           W1, b1, W2, b2, W3, b3, Wr1, br1, Wr2, br2,
           protein_atom_type, ligand_atom_type, protein_batch, ligand_batch):
    protein_pos = np.asarray(protein_pos, dtype=np.float32).reshape(B, P, 3)
    ligand_pos = np.asarray(ligand_pos, dtype=np.float32).reshape(B, L, 3)
    prot_emb = np.asarray(prot_emb, dtype=np.float32)
    lig_emb = np.asarray(lig_emb, dtype=np.float32)
    W1 = np.asarray(W1, dtype=np.float32)
    b1 = np.asarray(b1, dtype=np.float32)
    W2 = np.asarray(W2, dtype=np.float32)
    W3 = np.asarray(W3, dtype=np.float32)
    Wr1 = np.asarray(Wr1, dtype=np.float32)
    br1 = np.asarray(br1, dtype=np.float32).reshape(H)
    Wr2 = np.asarray(Wr2, dtype=np.float32).reshape(H)
    br2 = float(np.asarray(br2, dtype=np.float32).reshape(()))
    ptype = np.asarray(protein_atom_type).reshape(B, P)
    ltype = np.asarray(ligand_atom_type).reshape(B, L)

    # fused stage-1 weight: z1 = W1z.T @ [rb; onehot_p; onehot_l]
    PA = prot_emb @ W1[0:H, :]
    LA = lig_emb @ W1[H:2 * H, :] + b1[None, :]
    W1z = np.concatenate([W1[2 * H:2 * H + RB, :], PA, LA],
                         axis=0).astype(np.float16)
    wbb = np.concatenate([W2, W3], axis=1).astype(np.float16)
    fbb = np.ascontiguousarray(
        np.stack([b2.reshape(H), b3.reshape(H)], axis=1).astype(np.float32))

    centers = np.linspace(0.0, CUTOFF, RB, dtype=np.float32)

    # per-complex valid pairs -> global list of 512-pair tiles
    full_tiles = []     # (complex, rhs [K1,512] fp32) with no pad columns
    padded_tiles = []   # each complex's final, partially-padded tile
    cnts = []
    for b in range(B):
        diff = protein_pos[b][:, None, :] - ligand_pos[b][None, :, :]
        dist = np.sqrt((diff * diff).sum(-1, dtype=np.float32))
        pi, li = np.nonzero(dist < np.float32(CUTOFF))
        cnt = len(pi)
        cnts.append(cnt)
        dv = dist[pi, li]
        rhs = np.zeros((K1, ((cnt + 511) // 512) * 512), dtype=np.float32)
        rhs[0:RB, :cnt] = np.exp(
            -0.5 * ((dv[:, None] - centers[None, :]) / WIDTH) ** 2).T
        rhs[RB + ptype[b][pi], np.arange(cnt)] = 1.0
        rhs[RB + NPT + ltype[b][li], np.arange(cnt)] = 1.0
        nt_b = rhs.shape[1] // 512
        for s in range(nt_b):
            t = (b, rhs[:, 512 * s:512 * (s + 1)])
            if s == nt_b - 1 and cnt % 512 != 0:
                padded_tiles.append(t)
            else:
                full_tiles.append(t)

    ntot = len(full_tiles) + len(padded_tiles)
    nt = max(NT_DEFAULT, (ntot + N_CORES - 1) // N_CORES)
    # every core's LAST slot must contain a pad at column 511 (cvec source):
    # full tiles round-robin first, then one padded tile per core at the end;
    # any remaining slots become all-pad filler tiles.
    while True:
        core_tiles = [[] for _ in range(N_CORES)]
        core_pad = [False] * N_CORES
        for i, tb in enumerate(full_tiles):
            core_tiles[i % N_CORES].append(tb)
        for i, tb in enumerate(padded_tiles):
            core_tiles[i % N_CORES].append(tb)
            core_pad[i % N_CORES] = True
        ok = all(len(ct) <= nt and (len(ct) < nt or core_pad[k])
                 for k, ct in enumerate(core_tiles))
        if ok:
            break
        nt += 1
    npair = nt * 512

    in_maps = []
    tilemap = []                 # per core: complex id per tile (-1 = pad)
    for k in range(N_CORES):
        rhs = np.zeros((K1, npair), dtype=np.float32)
        cmap = []
        for s, (b, rt) in enumerate(core_tiles[k]):
            rhs[:, 512 * s:512 * (s + 1)] = rt
            cmap.append(b)
        cmap += [-1] * (nt - len(cmap))
        tilemap.append(cmap)
        in_maps.append({"rhsG": rhs.astype(np.float16), "wa": W1z,
                        "wb": wbb, "fb": fbb})

    nc = _get_nc(nt)
    res = bass_utils.run_bass_kernel_spmd(nc, in_maps,
                                          core_ids=list(range(N_CORES)))

    # unshard: attribute tile sums to complexes, remove pad contributions,
    # mean + tiny fp32 scoring head
    cvec = res.results[0]["cvec"][:, 0]
    tot = np.zeros((B, H), dtype=np.float64)
    ntiles_b = np.zeros(B, dtype=np.int64)
    for k in range(N_CORES):
        acc = res.results[k]["acc"]
        for s, b in enumerate(tilemap[k]):
            if b >= 0:
                tot[b] += acc[:, s]
                ntiles_b[b] += 1
    out = np.zeros(B, dtype=np.float32)
    for b in range(B):
        cnt = cnts[b]
        if cnt == 0:
            continue
        npad = ntiles_b[b] * 512 - cnt
        t2 = (tot[b] - npad * cvec).astype(np.float32)
        repr_ = (t2 / cnt).astype(np.float32)
        r1 = np.maximum(repr_ @ Wr1 + br1, 0.0)
        out[b] = r1 @ Wr2 + br2
    return out
